# revision 8
# baseline (speedup 1.0000x reference)
"""Trainium2 Bass kernel: batched 3x3 Lorenz-Jacobian Taylor matrix exponential.

Per element (u, x1, x2) = x[n, :]:
    A = dt * [[-10, 10, 0], [28, -1, -u], [0, u, -8/3]]
    F = sum_{j=0..5} A^j / j!          (truncated Taylor expm)
    y = F @ x

Structure exploited: every entry of F is an even or odd polynomial in u, and
F[2][1] = -F[1][2].  With v = u^2, w = v^2:
    F00 = a0 + a2 v
    F01 = b0 + b2 v (+ b4 w: dropped, contribution < 1.4e-6)
    F02 = u * (c1 + c3 v)
    F10 = d0 + d2 v (+ d4 w: dropped, < 2.9e-6)
    F11 = e0 + e2 v + e4 w
    F12 = u * (f1 + f3 v) (+ f5 w u: dropped, < 8e-7)
    F20 = u * (g1 + g3 v)
    F21 = -F12
    F22 = h0 + h2 v + h4 w

    y0 = ah*u + bh*x1 + ch*Q          Q = u*x2
    y1 = dh*u + eh*x1 + fh*Q
    y2 = gh*v + hh*x2 - fh*P          P = u*x1

(validated vs float64 reference: max scale-relative error 3.6e-7 in fp32)

Layout: batch sharded 8 ways; per core T tiles of [128 partitions, 3E floats]
(each partition row = E consecutive elements, x0 x1 x2 interleaved).
Component views are stride-3 APs; products/entries are dense [128, E] tiles.
Work split across ACT (squares + affine), DVE and GPSIMD (products, dots).
"""

import numpy as np
from contextlib import ExitStack

import concourse.bass as bass
import concourse.tile as tile
import concourse.mybir as mybir
from concourse.bass_utils import run_bass_kernel_spmd

DT = 0.02

# float64-exact Taylor coefficients of F entries (poly in v = u^2)
A0, A2 = 0.8679133685333335, -1.6824888888888892e-06
B0, B2 = 0.1827780802666667, -1.254811851851852e-05
C1, C3 = -0.0018440311802469136, 6.204444444444445e-08
D0, D2 = 0.5117786247466667, -3.513473185185185e-05
E0, E2, E4 = 1.0324136407733333, -0.00019737891358024691, 6.4444444444444465e-09
F1, F3 = -0.019630097558847738, 1.3003111111111115e-06
G1, G3 = 0.005163287304691359, -1.737244444444445e-07
H0, H2, H4 = 0.9480639384616735, -0.00019347448395061728, 6.400000000000002e-09

NCORES = 8
E_DEF = 978      # elements per partition row per tile (even; SBUF-fit)
T_DEF = 2        # tiles per core
B_IN = 2_000_000

FP32 = mybir.dt.float32
MULT = mybir.AluOpType.mult
ADD = mybir.AluOpType.add
SUB = mybir.AluOpType.subtract
IDENT = mybir.ActivationFunctionType.Identity


def build_nc(E=E_DEF, T=T_DEF):
    """Build the per-core Bass program: x[T,128,3E] -> y[T,128,3E]."""
    nc = bass.Bass("TRN2", target_bir_lowering=False, debug=False)
    # register const APs for ACT-activation bias operands
    for val in (A0, C1, D0, G1):
        t = nc.alloc_sbuf_tensor(f"const-f32-{val}", [128, 1], FP32)
        nc.gpsimd.memset(t.ap(), val)
        nc.const_aps.aps[(FP32, val)] = t.ap()
    nc.all_engine_barrier()
    x_d = nc.dram_tensor("x", [T, 128, 3 * E], FP32, kind="ExternalInput").ap()
    y_d = nc.dram_tensor("y", [T, 128, 3 * E], FP32, kind="ExternalOutput").ap()

    with tile.TileContext(nc) as tc, ExitStack() as ctx:
        xp = ctx.enter_context(tc.tile_pool(name="xp", bufs=2))
        pp = ctx.enter_context(tc.tile_pool(name="pp", bufs=2))
        ep = ctx.enter_context(tc.tile_pool(name="ep", bufs=2))
        mp = ctx.enter_context(tc.tile_pool(name="mp", bufs=2))

        for t in range(T):
            X = xp.tile([128, 3 * E], FP32)
            nc.sync.dma_start(X[:], x_d[t])
            u = X[:, 0:3 * E:3]
            x1 = X[:, 1:3 * E:3]
            x2 = X[:, 2:3 * E:3]

            v = pp.tile([128, E], FP32, tag="v")
            w = pp.tile([128, E], FP32, tag="w")
            P = pp.tile([128, E], FP32, tag="P")
            Q = pp.tile([128, E], FP32, tag="Q")
            # products
            nc.scalar.square(v[:], u)
            nc.scalar.square(w[:], v[:])
            nc.vector.tensor_tensor(P[:], u, x1, MULT)
            nc.vector.tensor_tensor(Q[:], u, x2, MULT)

            # entry polynomials in v (dense affine ops)
            ah = ep.tile([128, E], FP32, tag="ah")
            bh = ep.tile([128, E], FP32, tag="bh")
            ch = ep.tile([128, E], FP32, tag="ch")
            dh = ep.tile([128, E], FP32, tag="dh")
            eh = ep.tile([128, E], FP32, tag="eh")
            fh = ep.tile([128, E], FP32, tag="fh")
            gh = ep.tile([128, E], FP32, tag="gh")
            hh = ep.tile([128, E], FP32, tag="hh")
            nc.scalar.activation(ah[:], v[:], IDENT, bias=A0, scale=A2)
            nc.scalar.activation(ch[:], v[:], IDENT, bias=C1, scale=C3)
            nc.scalar.activation(dh[:], v[:], IDENT, bias=D0, scale=D2)
            nc.scalar.activation(gh[:], v[:], IDENT, bias=G1, scale=G3)
            nc.vector.tensor_scalar(bh[:], v[:], B2, B0, MULT, ADD)
            nc.vector.tensor_scalar(fh[:], v[:], F3, F1, MULT, ADD)
            nc.vector.tensor_scalar(eh[:], v[:], E2, E0, MULT, ADD)
            nc.vector.scalar_tensor_tensor(eh[:], w[:], E4, eh[:], MULT, ADD)
            nc.vector.tensor_scalar(hh[:], v[:], H2, H0, MULT, ADD)
            nc.vector.scalar_tensor_tensor(hh[:], w[:], H4, hh[:], MULT, ADD)

            # dot products: m tiles
            m1 = mp.tile([128, E], FP32, tag="m1")
            m2 = mp.tile([128, E], FP32, tag="m2")
            m3 = mp.tile([128, E], FP32, tag="m3")
            m4 = mp.tile([128, E], FP32, tag="m4")
            m5 = mp.tile([128, E], FP32, tag="m5")
            m6 = mp.tile([128, E], FP32, tag="m6")
            m7 = mp.tile([128, E], FP32, tag="m7")
            m8 = mp.tile([128, E], FP32, tag="m8")
            m9 = mp.tile([128, E], FP32, tag="m9")
            nc.vector.tensor_tensor(m1[:], ah[:], u, MULT)
            nc.vector.tensor_tensor(m2[:], bh[:], x1, MULT)
            nc.gpsimd.tensor_tensor(m3[:], ch[:], Q[:], MULT)
            nc.vector.tensor_tensor(m4[:], dh[:], u, MULT)
            nc.vector.tensor_tensor(m5[:], eh[:], x1, MULT)
            nc.gpsimd.tensor_tensor(m6[:], fh[:], Q[:], MULT)
            nc.gpsimd.tensor_tensor(m7[:], gh[:], v[:], MULT)
            nc.vector.tensor_tensor(m8[:], hh[:], x2, MULT)
            nc.gpsimd.tensor_tensor(m9[:], fh[:], P[:], MULT)

            # sums (in-place into m1/m4/m7), then final strided writes into X
            nc.gpsimd.tensor_tensor(m1[:], m1[:], m2[:], ADD)
            nc.gpsimd.tensor_tensor(m4[:], m4[:], m5[:], ADD)
            nc.gpsimd.tensor_tensor(m7[:], m7[:], m8[:], ADD)
            nc.vector.tensor_tensor(X[:, 0:3 * E:3], m1[:], m3[:], ADD)
            nc.vector.tensor_tensor(X[:, 1:3 * E:3], m4[:], m6[:], ADD)
            nc.vector.tensor_tensor(X[:, 2:3 * E:3], m7[:], m9[:], SUB)

            nc.sync.dma_start(y_d[t], X[:])

    _fix_tsp_waits(nc)
    return nc


def _fix_tsp_waits(nc):
    """Several TPB instruction encodings (S2S2D2_STT, pool S3S3D3_TT, ...)
    have a single sync-wait slot; Tile may attach several.  Hoist
    all-but-one wait onto same-engine nops inserted immediately before."""
    eng_map = {
        mybir.EngineType.DVE: nc.vector,
        mybir.EngineType.Activation: nc.scalar,
        mybir.EngineType.Pool: nc.gpsimd,
        mybir.EngineType.PE: nc.tensor,
        mybir.EngineType.SP: nc.sync,
    }
    for blk in nc.m.functions[0].blocks:
        i = 0
        while i < len(blk.instructions):
            ins = blk.instructions[i]
            if not isinstance(ins, mybir.InstNoOp) and ins.sync_info:
                waits = list(ins.sync_info.on_wait)
                if len(waits) > 1:
                    extra, keep = waits[:-1], waits[-1:]
                    ins.sync_info.on_wait = keep
                    for w in extra:
                        eng_map[ins.engine].nop()
                        nop = nc.m.functions[0].blocks[-1].instructions.pop()
                        assert isinstance(nop, mybir.InstNoOp)
                        nop.sync_info = mybir.SyncInfo(on_wait=[w], on_update=[])
                        blk.instructions.insert(i, nop)
                        i += 1
            i += 1


_CACHE = {}


def _get_nc(E, T):
    key = (E, T)
    if key not in _CACHE:
        _CACHE[key] = build_nc(E, T)
    return _CACHE[key]


def kernel(x: np.ndarray) -> np.ndarray:
    E, T = E_DEF, T_DEF
    n_pc = 128 * E * T                  # elements per core
    b_pad = NCORES * n_pc
    B = x.shape[0]
    assert x.shape[1] == 3 and b_pad >= B

    nc = _get_nc(E, T)
    xp = np.zeros((b_pad, 3), dtype=np.float32)
    xp[:B] = x
    shards = xp.reshape(NCORES, T, 128, 3 * E)
    in_maps = [{"x": shards[c]} for c in range(NCORES)]
    res = run_bass_kernel_spmd(nc, in_maps, list(range(NCORES)))
    y = np.concatenate([r["y"].reshape(n_pc, 3) for r in res.results], axis=0)
    return y[:B]


# revision 13
# speedup vs baseline: 95.9251x; 95.9251x over previous
"""Trainium2 Bass kernel: batched 3x3 Lorenz-Jacobian Taylor matrix exponential.

Per element (u, x1, x2) = x[n, :]:
    A = dt * [[-10, 10, 0], [28, -1, -u], [0, u, -8/3]]
    F = sum_{j=0..5} A^j / j!          (truncated Taylor expm)
    y = F @ x

Structure exploited: every entry of F is an even or odd polynomial in u, and
F[2][1] = -F[1][2].  With v = u^2, w = v^2:
    F00 = a0 + a2 v
    F01 = b0 + b2 v (+ b4 w: dropped, contribution < 1.4e-6)
    F02 = u * (c1 + c3 v)
    F10 = d0 + d2 v (+ d4 w: dropped, < 2.9e-6)
    F11 = e0 + e2 v + e4 w
    F12 = u * (f1 + f3 v) (+ f5 w u: dropped, < 8e-7)
    F20 = u * (g1 + g3 v)
    F21 = -F12
    F22 = h0 + h2 v + h4 w

    y0 = ah*u + bh*x1 + ch*Q          Q = u*x2
    y1 = dh*u + eh*x1 + fh*Q
    y2 = gh*v + hh*x2 - fh*P          P = u*x1

(validated vs float64 reference: max scale-relative error 3.6e-7 in fp32)

Layout: batch sharded 8 ways; per core T tiles of [128 partitions, 3E floats]
(each partition row = E consecutive elements, x0 x1 x2 interleaved).
Component views are stride-3 APs; products/entries are dense [128, E] tiles.
Work split across ACT (squares + affine), DVE and GPSIMD (products, dots).
"""

import numpy as np
from contextlib import ExitStack

import concourse.bass as bass
import concourse.tile as tile
import concourse.mybir as mybir
from concourse.bass_utils import run_bass_kernel_spmd

DT = 0.02

# float64-exact Taylor coefficients of F entries (poly in v = u^2)
A0, A2 = 0.8679133685333335, -1.6824888888888892e-06
B0, B2 = 0.1827780802666667, -1.254811851851852e-05
C1, C3 = -0.0018440311802469136, 6.204444444444445e-08
D0, D2 = 0.5117786247466667, -3.513473185185185e-05
E0, E2, E4 = 1.0324136407733333, -0.00019737891358024691, 6.4444444444444465e-09
F1, F3 = -0.019630097558847738, 1.3003111111111115e-06
G1, G3 = 0.005163287304691359, -1.737244444444445e-07
H0, H2, H4 = 0.9480639384616735, -0.00019347448395061728, 6.400000000000002e-09

NCORES = 8
E_DEF = 489      # elements per partition row per tile
T_DEF = 4        # tiles per core
B_IN = 2_000_000

FP32 = mybir.dt.float32
MULT = mybir.AluOpType.mult
ADD = mybir.AluOpType.add
SUB = mybir.AluOpType.subtract
IDENT = mybir.ActivationFunctionType.Identity


def make_pools(tc, ctx):
    return {
        "xp": ctx.enter_context(tc.tile_pool(name="xp", bufs=3)),
        "pp": ctx.enter_context(tc.tile_pool(name="pp", bufs=3)),
        "ep": ctx.enter_context(tc.tile_pool(name="ep", bufs=3)),
    }


def emit_tile(nc, pools, x_src, y_dst, E):
    """Emit the full per-tile dataflow: x_src -> y_dst ([128, 3E] DRAM APs).

    Dot products are written in place into dead entry/product tiles (no
    m-pool), keeping the footprint at 60E B/partition so bufs=3 fits.
    Engine assignment found by greedy TimelineSim search."""
    xp, pp, ep = pools["xp"], pools["pp"], pools["ep"]
    X = xp.tile([128, 3 * E], FP32, tag="X", name="X")
    nc.sync.dma_start(X[:], x_src)
    u = X[:, 0:3 * E:3]
    x1 = X[:, 1:3 * E:3]
    x2 = X[:, 2:3 * E:3]

    v = pp.tile([128, E], FP32, tag="v", name="v")
    w = pp.tile([128, E], FP32, tag="w", name="w")
    P = pp.tile([128, E], FP32, tag="P", name="P")
    Q = pp.tile([128, E], FP32, tag="Q", name="Q")
    # products
    nc.scalar.square(v[:], u)
    nc.scalar.square(w[:], v[:])
    nc.vector.tensor_tensor(P[:], u, x1, MULT)
    nc.vector.tensor_tensor(Q[:], u, x2, MULT)

    # entry polynomials in v (dense affine ops)
    ah = ep.tile([128, E], FP32, tag="ah", name="ah")
    bh = ep.tile([128, E], FP32, tag="bh", name="bh")
    ch = ep.tile([128, E], FP32, tag="ch", name="ch")
    dh = ep.tile([128, E], FP32, tag="dh", name="dh")
    eh = ep.tile([128, E], FP32, tag="eh", name="eh")
    fh = ep.tile([128, E], FP32, tag="fh", name="fh")
    gh = ep.tile([128, E], FP32, tag="gh", name="gh")
    hh = ep.tile([128, E], FP32, tag="hh", name="hh")
    nc.scalar.activation(ah[:], v[:], IDENT, bias=A0, scale=A2)
    nc.scalar.activation(bh[:], v[:], IDENT, bias=B0, scale=B2)
    nc.scalar.activation(ch[:], v[:], IDENT, bias=C1, scale=C3)
    nc.scalar.activation(dh[:], v[:], IDENT, bias=D0, scale=D2)
    nc.scalar.activation(gh[:], v[:], IDENT, bias=G1, scale=G3)
    nc.vector.tensor_scalar(fh[:], v[:], F3, F1, MULT, ADD)
    nc.scalar.activation(eh[:], v[:], IDENT, bias=E0, scale=E2)
    nc.vector.scalar_tensor_tensor(eh[:], w[:], E4, eh[:], MULT, ADD)
    nc.scalar.activation(hh[:], v[:], IDENT, bias=H0, scale=H2)
    nc.vector.scalar_tensor_tensor(hh[:], w[:], H4, hh[:], MULT, ADD)

    # dot products, in place into dead tiles
    nc.vector.tensor_tensor(ah[:], ah[:], u, MULT)       # m1 -> ah
    nc.vector.tensor_tensor(bh[:], bh[:], x1, MULT)      # m2 -> bh
    nc.gpsimd.tensor_tensor(ch[:], ch[:], Q[:], MULT)    # m3 -> ch
    nc.vector.tensor_tensor(dh[:], dh[:], u, MULT)       # m4 -> dh
    nc.vector.tensor_tensor(eh[:], eh[:], x1, MULT)      # m5 -> eh
    nc.vector.tensor_tensor(Q[:], fh[:], Q[:], MULT)     # m6 -> Q (after m3)
    nc.gpsimd.tensor_tensor(gh[:], gh[:], v[:], MULT)    # m7 -> gh
    nc.vector.tensor_tensor(hh[:], hh[:], x2, MULT)      # m8 -> hh
    nc.gpsimd.tensor_tensor(P[:], fh[:], P[:], MULT)     # m9 -> P

    # sums, then final strided writes back into X
    nc.gpsimd.tensor_tensor(ah[:], ah[:], bh[:], ADD)
    nc.gpsimd.tensor_tensor(dh[:], dh[:], eh[:], ADD)
    nc.gpsimd.tensor_tensor(gh[:], gh[:], hh[:], ADD)
    nc.vector.tensor_tensor(X[:, 0:3 * E:3], ah[:], ch[:], ADD)
    nc.vector.tensor_tensor(X[:, 1:3 * E:3], dh[:], Q[:], ADD)
    nc.vector.tensor_tensor(X[:, 2:3 * E:3], gh[:], P[:], SUB)

    nc.sync.dma_start(y_dst, X[:])


def build_nc(E=E_DEF, T=T_DEF):
    """Build the per-core Bass program: x[T,128,3E] -> y[T,128,3E]."""
    nc = bass.Bass("TRN2", target_bir_lowering=False, debug=False)
    # register const APs for ACT-activation bias operands
    for val in (A0, B0, C1, D0, E0, G1, H0):
        t = nc.alloc_sbuf_tensor(f"const-f32-{val}", [128, 1], FP32)
        nc.gpsimd.memset(t.ap(), val)
        nc.const_aps.aps[(FP32, val)] = t.ap()
    nc.all_engine_barrier()
    x_d = nc.dram_tensor("x", [T, 128, 3 * E], FP32, kind="ExternalInput").ap()
    y_d = nc.dram_tensor("y", [T, 128, 3 * E], FP32, kind="ExternalOutput").ap()

    with tile.TileContext(nc) as tc, ExitStack() as ctx:
        pools = make_pools(tc, ctx)
        for t in range(T):
            emit_tile(nc, pools, x_d[t], y_d[t], E)

    _fix_tsp_waits(nc)
    return nc


def _fix_tsp_waits(nc):
    """Several TPB instruction encodings (S2S2D2_STT, pool S3S3D3_TT, ...)
    have a single sync-wait slot; Tile may attach several.  Hoist
    all-but-one wait onto same-engine nops inserted immediately before."""
    eng_map = {
        mybir.EngineType.DVE: nc.vector,
        mybir.EngineType.Activation: nc.scalar,
        mybir.EngineType.Pool: nc.gpsimd,
        mybir.EngineType.PE: nc.tensor,
        mybir.EngineType.SP: nc.sync,
    }
    for blk in nc.m.functions[0].blocks:
        i = 0
        while i < len(blk.instructions):
            ins = blk.instructions[i]
            if ins.sync_info:
                waits = list(ins.sync_info.on_wait)
                if len(waits) > 1:
                    extra, keep = waits[:-1], waits[-1:]
                    ins.sync_info.on_wait = keep
                    for w in extra:
                        eng_map[ins.engine].nop()
                        nop = nc.m.functions[0].blocks[-1].instructions.pop()
                        assert isinstance(nop, mybir.InstNoOp)
                        nop.sync_info = mybir.SyncInfo(on_wait=[w], on_update=[])
                        blk.instructions.insert(i, nop)
                        i += 1
            i += 1


_CACHE = {}


def _get_nc(E, T):
    key = (E, T)
    if key not in _CACHE:
        _CACHE[key] = build_nc(E, T)
    return _CACHE[key]


def kernel(x: np.ndarray) -> np.ndarray:
    E, T = E_DEF, T_DEF
    n_pc = 128 * E * T                  # elements per core
    b_pad = NCORES * n_pc
    B = x.shape[0]
    assert x.shape[1] == 3 and b_pad >= B

    nc = _get_nc(E, T)
    xp = np.zeros((b_pad, 3), dtype=np.float32)
    xp[:B] = x
    shards = xp.reshape(NCORES, T, 128, 3 * E)
    in_maps = [{"x": shards[c]} for c in range(NCORES)]
    res = run_bass_kernel_spmd(nc, in_maps, list(range(NCORES)))
    y = np.concatenate([r["y"].reshape(n_pc, 3) for r in res.results], axis=0)
    return y[:B]


# revision 17
# speedup vs baseline: 105.3044x; 1.0978x over previous
"""Trainium2 Bass kernel: batched 3x3 Lorenz-Jacobian Taylor matrix exponential.

Per element (u, x1, x2) = x[n, :]:
    A = dt * [[-10, 10, 0], [28, -1, -u], [0, u, -8/3]]
    F = sum_{j=0..5} A^j / j!          (truncated Taylor expm)
    y = F @ x

Structure exploited: every entry of F is an even or odd polynomial in u, and
F[2][1] = -F[1][2].  With v = u^2, w = v^2:
    F00 = a0 + a2 v
    F01 = b0 + b2 v (+ b4 w: dropped, contribution < 1.4e-6)
    F02 = u * (c1 + c3 v)
    F10 = d0 + d2 v (+ d4 w: dropped, < 2.9e-6)
    F11 = e0 + e2 v (+ e4 w: dropped, < 1.9e-5)
    F12 = u * (f1 + f3 v) (+ f5 w u: dropped, < 8e-7)
    F20 = u * (g1 + g3 v)
    F21 = -F12
    F22 = h0 + h2 v (+ h4 w: dropped, < 1.9e-5)

    y0 = ah*u + bh*x1 + ch*Q          Q = u*x2
    y1 = dh*u + eh*x1 + fh*Q
    y2 = gh*v + hh*x2 - fh*P          P = u*x1

(validated vs float64 reference: max scale-relative error 1.4e-6 in fp32)

Layout: batch sharded 8 ways; per core T tiles of [128 partitions, 3E floats]
(each partition row = E consecutive elements, x0 x1 x2 interleaved).
Component views are stride-3 APs; products/entries are dense [128, E] tiles.
Work split across ACT (squares + affine), DVE and GPSIMD (products, dots).
"""

import numpy as np
from contextlib import ExitStack

import concourse.bass as bass
import concourse.tile as tile
import concourse.mybir as mybir
from concourse.bass_utils import run_bass_kernel_spmd

DT = 0.02

# float64-exact Taylor coefficients of F entries (poly in v = u^2)
A0, A2 = 0.8679133685333335, -1.6824888888888892e-06
B0, B2 = 0.1827780802666667, -1.254811851851852e-05
C1, C3 = -0.0018440311802469136, 6.204444444444445e-08
D0, D2 = 0.5117786247466667, -3.513473185185185e-05
E0, E2, E4 = 1.0324136407733333, -0.00019737891358024691, 6.4444444444444465e-09
F1, F3 = -0.019630097558847738, 1.3003111111111115e-06
G1, G3 = 0.005163287304691359, -1.737244444444445e-07
H0, H2, H4 = 0.9480639384616735, -0.00019347448395061728, 6.400000000000002e-09

NCORES = 8
E_DEF = 489      # elements per partition row per tile
T_DEF = 4        # tiles per core
B_IN = 2_000_000

FP32 = mybir.dt.float32
MULT = mybir.AluOpType.mult
ADD = mybir.AluOpType.add
SUB = mybir.AluOpType.subtract
IDENT = mybir.ActivationFunctionType.Identity


def make_pools(tc, ctx):
    return {
        "xp": ctx.enter_context(tc.tile_pool(name="xp", bufs=3)),
        "pp": ctx.enter_context(tc.tile_pool(name="pp", bufs=3)),
        "ep": ctx.enter_context(tc.tile_pool(name="ep", bufs=3)),
    }


def emit_tile(nc, pools, x_src, y_dst, E):
    """Emit the full per-tile dataflow: x_src -> y_dst ([128, 3E] DRAM APs).

    Dot products are written in place into dead entry/product tiles (no
    m-pool), keeping the footprint at 60E B/partition so bufs=3 fits.
    Engine assignment found by greedy TimelineSim search."""
    xp, pp, ep = pools["xp"], pools["pp"], pools["ep"]
    X = xp.tile([128, 3 * E], FP32, tag="X", name="X")
    nc.sync.dma_start(X[:], x_src)
    u = X[:, 0:3 * E:3]
    x1 = X[:, 1:3 * E:3]
    x2 = X[:, 2:3 * E:3]

    v = pp.tile([128, E], FP32, tag="v", name="v")
    P = pp.tile([128, E], FP32, tag="P", name="P")
    Q = pp.tile([128, E], FP32, tag="Q", name="Q")
    # products
    nc.scalar.square(v[:], u)
    nc.vector.tensor_tensor(P[:], u, x1, MULT)
    nc.vector.tensor_tensor(Q[:], u, x2, MULT)

    # entry polynomials in v (dense affine ops)
    ah = ep.tile([128, E], FP32, tag="ah", name="ah")
    bh = ep.tile([128, E], FP32, tag="bh", name="bh")
    ch = ep.tile([128, E], FP32, tag="ch", name="ch")
    dh = ep.tile([128, E], FP32, tag="dh", name="dh")
    eh = ep.tile([128, E], FP32, tag="eh", name="eh")
    fh = ep.tile([128, E], FP32, tag="fh", name="fh")
    gh = ep.tile([128, E], FP32, tag="gh", name="gh")
    hh = ep.tile([128, E], FP32, tag="hh", name="hh")
    nc.scalar.activation(ah[:], v[:], IDENT, bias=A0, scale=A2)
    nc.scalar.activation(bh[:], v[:], IDENT, bias=B0, scale=B2)
    nc.scalar.activation(ch[:], v[:], IDENT, bias=C1, scale=C3)
    nc.scalar.activation(dh[:], v[:], IDENT, bias=D0, scale=D2)
    nc.scalar.activation(gh[:], v[:], IDENT, bias=G1, scale=G3)
    nc.vector.tensor_scalar(fh[:], v[:], F3, F1, MULT, ADD)
    nc.scalar.activation(eh[:], v[:], IDENT, bias=E0, scale=E2)
    nc.scalar.activation(hh[:], v[:], IDENT, bias=H0, scale=H2)

    # dot products, in place into dead tiles
    nc.vector.tensor_tensor(ah[:], ah[:], u, MULT)       # m1 -> ah
    nc.vector.tensor_tensor(bh[:], bh[:], x1, MULT)      # m2 -> bh
    nc.gpsimd.tensor_tensor(ch[:], ch[:], Q[:], MULT)    # m3 -> ch
    nc.vector.tensor_tensor(dh[:], dh[:], u, MULT)       # m4 -> dh
    nc.vector.tensor_tensor(eh[:], eh[:], x1, MULT)      # m5 -> eh
    nc.vector.tensor_tensor(Q[:], fh[:], Q[:], MULT)     # m6 -> Q (after m3)
    nc.gpsimd.tensor_tensor(gh[:], gh[:], v[:], MULT)    # m7 -> gh
    nc.vector.tensor_tensor(hh[:], hh[:], x2, MULT)      # m8 -> hh
    nc.gpsimd.tensor_tensor(P[:], fh[:], P[:], MULT)     # m9 -> P

    # sums, then final strided writes back into X
    nc.gpsimd.tensor_tensor(ah[:], ah[:], bh[:], ADD)
    nc.gpsimd.tensor_tensor(dh[:], dh[:], eh[:], ADD)
    nc.gpsimd.tensor_tensor(gh[:], gh[:], hh[:], ADD)
    nc.vector.tensor_tensor(X[:, 0:3 * E:3], ah[:], ch[:], ADD)
    nc.vector.tensor_tensor(X[:, 1:3 * E:3], dh[:], Q[:], ADD)
    nc.vector.tensor_tensor(X[:, 2:3 * E:3], gh[:], P[:], SUB)

    nc.sync.dma_start(y_dst, X[:])


def build_nc(E=E_DEF, T=T_DEF):
    """Build the per-core Bass program: x[T,128,3E] -> y[T,128,3E]."""
    nc = bass.Bass("TRN2", target_bir_lowering=False, debug=False)
    # register const APs for ACT-activation bias operands
    for val in (A0, B0, C1, D0, E0, G1, H0):
        t = nc.alloc_sbuf_tensor(f"const-f32-{val}", [128, 1], FP32)
        nc.gpsimd.memset(t.ap(), val)
        nc.const_aps.aps[(FP32, val)] = t.ap()
    nc.all_engine_barrier()
    x_d = nc.dram_tensor("x", [T, 128, 3 * E], FP32, kind="ExternalInput").ap()
    y_d = nc.dram_tensor("y", [T, 128, 3 * E], FP32, kind="ExternalOutput").ap()

    with tile.TileContext(nc) as tc, ExitStack() as ctx:
        pools = make_pools(tc, ctx)
        for t in range(T):
            emit_tile(nc, pools, x_d[t], y_d[t], E)

    _fix_tsp_waits(nc)
    return nc


def _fix_tsp_waits(nc):
    """Several TPB instruction encodings (S2S2D2_STT, pool S3S3D3_TT, ...)
    have a single sync-wait slot; Tile may attach several.  Hoist
    all-but-one wait onto same-engine nops inserted immediately before."""
    eng_map = {
        mybir.EngineType.DVE: nc.vector,
        mybir.EngineType.Activation: nc.scalar,
        mybir.EngineType.Pool: nc.gpsimd,
        mybir.EngineType.PE: nc.tensor,
        mybir.EngineType.SP: nc.sync,
    }
    for blk in nc.m.functions[0].blocks:
        i = 0
        while i < len(blk.instructions):
            ins = blk.instructions[i]
            if ins.sync_info:
                waits = list(ins.sync_info.on_wait)
                if len(waits) > 1:
                    extra, keep = waits[:-1], waits[-1:]
                    ins.sync_info.on_wait = keep
                    for w in extra:
                        eng_map[ins.engine].nop()
                        nop = nc.m.functions[0].blocks[-1].instructions.pop()
                        assert isinstance(nop, mybir.InstNoOp)
                        nop.sync_info = mybir.SyncInfo(on_wait=[w], on_update=[])
                        blk.instructions.insert(i, nop)
                        i += 1
            i += 1


_CACHE = {}


def _get_nc(E, T):
    key = (E, T)
    if key not in _CACHE:
        _CACHE[key] = build_nc(E, T)
    return _CACHE[key]


def kernel(x: np.ndarray) -> np.ndarray:
    E, T = E_DEF, T_DEF
    n_pc = 128 * E * T                  # elements per core
    b_pad = NCORES * n_pc
    B = x.shape[0]
    assert x.shape[1] == 3 and b_pad >= B

    nc = _get_nc(E, T)
    xp = np.zeros((b_pad, 3), dtype=np.float32)
    xp[:B] = x
    shards = xp.reshape(NCORES, T, 128, 3 * E)
    in_maps = [{"x": shards[c]} for c in range(NCORES)]
    res = run_bass_kernel_spmd(nc, in_maps, list(range(NCORES)))
    y = np.concatenate([r["y"].reshape(n_pc, 3) for r in res.results], axis=0)
    return y[:B]
